# revision 3
# baseline (speedup 1.0000x reference)
"""Trainium2 Bass kernel for nn_MetaNetLinearizedModel (v5).

kv2 design with w8 in 3 groups of 49 (6.4MB DMAs, fewer fixed costs);
only the final group is sub-sliced (28/14/7 chunks) to kill the exposed
last-group matmul trail.  Single sync ring, z bytes first.

Stream: W1 bf16 + dW1 fp8e4m3(x256), host-pretransposed partition-major
layout -> fully contiguous group DMAs on ONE queue (sync), W1 bytes first.
wb: 3 groups x 49 chunks (3.2MB, bufs=3 resident); w8: 7 groups x 21 chunks
(2.75MB, bufs=4).  3 PE lanes (z cg0 / u01 cg2 / u23 cg3) overlap.

Reduce: bf16 AllGather (z early + u late) + PE P-matrix reduction instead of
AllReduce.  The z-AG fires mid-stream; the z-dependent tail front (h, mask,
base, coefs, e-blocks, o_pre) hides under the u stream.  After stream end
only: S_u assembly, AG_u, u-reduce, G-combine, pg, final add, out DMA.

Small/tail DMAs ride the scalar queue (never blocked behind the weight
stream); consts load on gpsimd at t=0.
"""

from contextlib import nullcontext

import numpy as np
import ml_dtypes

import concourse.bass as bass
import concourse.mybir as mybir
import concourse.tile as tile
from concourse import bacc
from concourse.bass_utils import run_bass_kernel_spmd

BF16 = ml_dtypes.bfloat16
FP8 = ml_dtypes.float8_e4m3

N_CORES = 8
B = 8
D_IN = 3 * 224 * 224
FEAT = 256
HID = 64
T = 4
KC = D_IN // N_CORES          # 18816
NK = KC // 128                # 147
GB = 49                       # wb chunks per group (3 groups)
NGB = NK // GB
G8 = 49                       # w8 chunks per group (3 groups)
NG8 = NK // G8
DW1_SCALE = 256.0
SRED = 5 * FEAT

F32 = mybir.dt.float32
BF = mybir.dt.bfloat16
F8 = mybir.dt.float8e4
AOT = mybir.AluOpType

_CACHE = {}


class _Env:
    pass


def _emit_dmas(nc, e):
    e.wb = []
    for g in range(NGB):
        wb = e.wpool.tile([128, GB, FEAT], BF, tag="wb", name=f"wb{g}")
        nc.sync.dma_start(
            wb[:], e.wbf_d[:, g * GB * FEAT:(g + 1) * GB * FEAT]
            .rearrange("p (c n) -> p c n", c=GB))
        e.wb.append(wb)
    e.w8 = []
    for g in range(NG8):
        w8 = e.wpool8.tile([128, G8, 4 * FEAT], F8, tag="w8", name=f"w8{g}")
        if g < NG8 - 1:
            nc.sync.dma_start(
                w8[:], e.w8_d[:, g * G8 * 4 * FEAT:(g + 1) * G8 * 4 * FEAT]
                .rearrange("p (c n) -> p c n", c=G8))
        else:
            for lo, hi in ((0, 28), (28, 42), (42, 49)):
                c0 = g * G8 + lo
                nc.sync.dma_start(
                    w8[:, lo:hi, :],
                    e.w8_d[:, c0 * 4 * FEAT:(g * G8 + hi) * 4 * FEAT]
                    .rearrange("p (c n) -> p c n", c=hi - lo))
        e.w8.append(w8)


def _emit_zmms(nc, e):
    for g in range(NGB):
        for c in range(GB):
            k = g * GB + c
            nc.tensor.matmul(e.bk0[0:B, 0:256], e.xhi[:, k, :],
                             e.wb[g][:, c, :], start=(k == 0),
                             stop=(k == NK - 1), tile_position=(0, 0))


def _emit_umms(nc, e, glo, ghi):
    for g in range(glo, ghi):
        for c in range(G8):
            k = g * G8 + c
            st = (k == 0)
            sp = (k == NK - 1)
            xh = e.xhi[:, k, :]
            nc.tensor.matmul(e.bk2[64:64 + B, :], xh, e.w8[g][:, c, 0:512],
                             start=st, stop=sp, tile_position=(0, 64))
            nc.tensor.matmul(e.bk3[96:96 + B, :], xh, e.w8[g][:, c, 512:1024],
                             start=st, stop=sp, tile_position=(0, 96))


def _emit_Sz(nc, e):
    nc.vector.tensor_add(e.Sz[:], e.bk0[0:B, 0:256], e.bias8[:, 0:256])


def _emit_Su(nc, e):
    nc.vector.scalar_tensor_tensor(
        e.Su[:, 0:512], e.bk2[64:64 + B, :], 1.0 / DW1_SCALE,
        e.bias8[:, 256:768], op0=AOT.mult, op1=AOT.add)
    nc.vector.scalar_tensor_tensor(
        e.Su[:, 512:1024], e.bk3[96:96 + B, :], 1.0 / DW1_SCALE,
        e.bias8[:, 768:1280], op0=AOT.mult, op1=AOT.add)


def _emit_agz(nc, e, it=""):
    cin = e.dram.tile([B, FEAT], BF, tag="cinz", name=f"cinz{it}")
    cout = e.dram.tile([N_CORES * B, FEAT], BF, tag="coutz",
                       name=f"coutz{it}")
    nc.scalar.dma_start(cin[:], e.Sz[:])
    nc.gpsimd.collective_compute(
        "AllGather", AOT.bypass,
        replica_groups=[list(range(N_CORES))],
        ins=[cin.opt()], outs=[cout.opt()])
    e.Gz = e.sb.tile([N_CORES * B, FEAT], BF, tag="Gz", name=f"Gz{it}")
    nc.scalar.dma_start(e.Gz[:], cout[:])


def _emit_agu(nc, e, it=""):
    cin = e.dram.tile([B, 4 * FEAT], BF, tag="cinu", name=f"cinu{it}")
    cout = e.dram.tile([N_CORES * B, 4 * FEAT], BF, tag="coutu",
                       name=f"coutu{it}")
    nc.scalar.dma_start(cin[:], e.Su[:])
    nc.gpsimd.collective_compute(
        "AllGather", AOT.bypass,
        replica_groups=[list(range(N_CORES))],
        ins=[cin.opt()], outs=[cout.opt()])
    e.Gu = e.sb.tile([N_CORES * B, 4 * FEAT], BF, tag="Gu", name=f"Gu{it}")
    nc.scalar.dma_start(e.Gu[:], cout[:])


def _emit_front(nc, e):
    """z-reduce + everything that depends only on z (+ consts)."""
    sb, sb2 = e.sb, e.sb2
    Rz = e.ps_rz.tile([128, 512], F32, tag="Rz", name="Rz")
    nc.tensor.matmul(Rz[0:B, 0:256], e.Pmat[:], e.Gz[:], start=True,
                     stop=True)
    z = Rz[0:B, 0:256]
    h = sb.tile([B, FEAT], F32, tag="h", name="h")
    nc.vector.tensor_scalar_max(h[:], z, 0.0)
    mask = sb.tile([B, FEAT], F32, tag="mask", name="mask")
    nc.vector.tensor_scalar(mask[:], z, 0.0, None, op0=AOT.is_gt)
    e.mask = mask

    def tr2(dst, src):
        for c in range(2):
            tp = e.ps_tp.tile([128, B], F32, tag="tp", name="tp")
            nc.tensor.transpose(tp[:], src[:, c * 128:(c + 1) * 128],
                                e.id8[:])
            nc.vector.tensor_copy(dst[:, c, :], tp[:])

    e.tr2 = tr2
    hT = sb.tile([128, 2, B], F32, tag="hT", name="hT")
    tr2(hT, h[:])
    hTb = sb.tile([128, 2, B], BF, tag="hTb", name="hTb")
    nc.vector.tensor_copy(hTb[:], hT[:])

    pb = e.ps2.tile([B, FEAT], F32, tag="pp", name="pb")
    nc.tensor.matmul(pb[:], hT[:, 0, :], e.w2t[:, 0, :],
                     start=True, stop=False)
    nc.tensor.matmul(pb[:], hT[:, 1, :], e.w2t[:, 1, :],
                     start=False, stop=False)
    nc.tensor.matmul(pb[:], e.ones1[:], e.brow[:, 0:256],
                     start=False, stop=True)
    base = sb.tile([B, FEAT], F32, tag="base", name="base")
    nc.vector.tensor_copy(base[:], pb[:])

    bT = sb.tile([128, 2, B], F32, tag="bT", name="bT")
    tr2(bT, base[:])
    pm = e.ps2.tile([B, HID], F32, tag="pp", name="pm")
    nc.tensor.matmul(pm[:], bT[:, 0, :], e.mw1t[:, 0, :],
                     start=True, stop=False)
    nc.tensor.matmul(pm[:], bT[:, 1, :], e.mw1t[:, 1, :],
                     start=False, stop=False)
    nc.tensor.matmul(pm[:], e.ones1[:], e.brow[:, 256:320],
                     start=False, stop=True)
    m1 = sb.tile([B, HID], F32, tag="m1", name="m1")
    nc.vector.tensor_scalar_max(m1[:], pm[:], 0.0)

    tpm = e.ps_tp.tile([128, B], F32, tag="tp", name="tpm")
    nc.tensor.transpose(tpm[0:HID, :], m1[:], e.id8[:])
    m1t = sb.tile([HID, B], F32, tag="m1t", name="m1t")
    nc.vector.tensor_copy(m1t[:], tpm[0:HID, :])

    pc = e.ps2.tile([B, T], F32, tag="pp", name="pc")
    nc.tensor.matmul(pc[:], m1t[:], e.mw2t[:], start=True, stop=False)
    nc.tensor.matmul(pc[:], e.ones1[:], e.brow[:, 320:324],
                     start=False, stop=True)
    coefs = sb.tile([B, T], F32, tag="coefs", name="coefs")
    nc.vector.tensor_copy(coefs[:], pc[:])
    e.coefs = coefs

    # e-blocks: h @ dW2_t.T + db2_t  (bf16; feeds only coef-scaled terms)
    pe1 = e.ps_e.tile([B, 512], F32, tag="pe", name="pe1")
    nc.tensor.matmul(pe1[:], hTb[:, 0, :], e.dw2[:, 0, 0:512],
                     start=True, stop=False)
    nc.tensor.matmul(pe1[:], hTb[:, 1, :], e.dw2[:, 1, 0:512],
                     start=False, stop=False)
    nc.tensor.matmul(pe1[:], e.ones1b[:], e.browb[:, 0:512],
                     start=False, stop=True)
    pe2 = e.ps_e.tile([B, 512], F32, tag="pe", name="pe2")
    nc.tensor.matmul(pe2[:], hTb[:, 0, :], e.dw2[:, 0, 512:1024],
                     start=True, stop=False)
    nc.tensor.matmul(pe2[:], hTb[:, 1, :], e.dw2[:, 1, 512:1024],
                     start=False, stop=False)
    nc.tensor.matmul(pe2[:], e.ones1b[:], e.browb[:, 512:1024],
                     start=False, stop=True)

    # o_pre = base + sum_t coefs_t * e_t   (off the critical tail)
    o = base
    for t in range(T):
        pe = pe1 if t < 2 else pe2
        off = 256 * (t % 2)
        o2 = sb2.tile([B, FEAT], F32, tag="oacc", name=f"op{t}")
        nc.vector.scalar_tensor_tensor(
            o2[:], pe[:, off:off + 256], coefs[:, t:t + 1], o[:],
            op0=AOT.mult, op1=AOT.add)
        o = o2
    e.o_pre = o


def _emit_back(nc, e, out_d):
    """u-reduce + G-combine + pg + final; the only post-stream tail."""
    sb, sb2 = e.sb, e.sb2
    Ru1 = e.ps_ru.tile([128, 512], F32, tag="Ru1", name="Ru1")
    nc.tensor.matmul(Ru1[0:B, :], e.Pmat[:], e.Gu[:, 0:512],
                     start=True, stop=True)
    Ru2 = e.ps_ru.tile([128, 512], F32, tag="Ru2", name="Ru2")
    nc.tensor.matmul(Ru2[0:B, :], e.Pmat[:], e.Gu[:, 512:1024],
                     start=True, stop=True)

    coefs = e.coefs
    ga = sb2.tile([B, FEAT], F32, tag="gacc", name="ga0")
    nc.vector.tensor_scalar_mul(ga[:], Ru1[0:B, 0:256], coefs[:, 0:1])
    for t, src in ((1, Ru1[0:B, 256:512]), (2, Ru2[0:B, 0:256]),
                   (3, Ru2[0:B, 256:512])):
        ga2 = sb2.tile([B, FEAT], F32, tag="gacc", name=f"ga{t}")
        nc.vector.scalar_tensor_tensor(
            ga2[:], src, coefs[:, t:t + 1], ga[:],
            op0=AOT.mult, op1=AOT.add)
        ga = ga2
    G = sb.tile([B, FEAT], F32, tag="G", name="G")
    nc.vector.tensor_mul(G[:], ga[:], e.mask[:])

    gT = sb.tile([128, 2, B], F32, tag="gT", name="gT")
    e.tr2(gT, G[:])
    pg = e.ps2.tile([B, FEAT], F32, tag="pp", name="pg")
    nc.tensor.matmul(pg[:], gT[:, 0, :], e.w2t[:, 0, :],
                     start=True, stop=False)
    nc.tensor.matmul(pg[:], gT[:, 1, :], e.w2t[:, 1, :],
                     start=False, stop=True)

    o = sb.tile([B, FEAT], F32, tag="ofin", name="ofin")
    nc.vector.tensor_add(o[:], e.o_pre[:], pg[:])
    nc.scalar.dma_start(out_d[:], o[:])
    e.o_fin = o


def _build(reps1=1, reps2=1, body=1, shots=1, wbufs=2, tails=1):
    nc = bacc.Bacc("TRN2", target_bir_lowering=False, debug=False,
                   num_devices=N_CORES)

    e = _Env()
    e.wbf_d = nc.dram_tensor("wbf", [128, NK * FEAT], BF,
                             kind="ExternalInput")
    e.w8_d = nc.dram_tensor("w8", [128, NK * 4 * FEAT], F8,
                            kind="ExternalInput")
    xhi_d = nc.dram_tensor("xhi", [128, NK, B], BF, kind="ExternalInput")
    w2t_d = nc.dram_tensor("w2t", [FEAT, FEAT], F32, kind="ExternalInput")
    mw1t_d = nc.dram_tensor("mw1t", [FEAT, HID], F32, kind="ExternalInput")
    dw2_d = nc.dram_tensor("dw2cat", [FEAT, T * FEAT], BF,
                           kind="ExternalInput")
    browb_d = nc.dram_tensor("browb", [1, FEAT + HID + T + T * FEAT], BF,
                             kind="ExternalInput")
    mw2t_d = nc.dram_tensor("mw2t", [HID, T], F32, kind="ExternalInput")
    brow_d = nc.dram_tensor("brow", [1, FEAT + HID + T + T * FEAT], F32,
                            kind="ExternalInput")
    bias8_d = nc.dram_tensor("bias8", [B, SRED], F32, kind="ExternalInput")
    id8_d = nc.dram_tensor("ident8", [B, B], F32, kind="ExternalInput")
    pmat_d = nc.dram_tensor("pmat", [N_CORES * B, B], BF,
                            kind="ExternalInput")
    out_d = nc.dram_tensor("out", [B, FEAT], F32, kind="ExternalOutput")

    with tile.TileContext(nc) as tc:
        with (
            tc.tile_pool(name="const", bufs=1) as cpool,
            tc.tile_pool(name="wstream", bufs=3) as e_wpool,
            tc.tile_pool(name="wstream8", bufs=wbufs) as e_wpool8,
            tc.tile_pool(name="sb", bufs=1) as e_sb,
            tc.tile_pool(name="sb2", bufs=2) as e_sb2,
            tc.tile_pool(name="dram", bufs=1, space="DRAM") as e_dram,
        ):
            e.wpool, e.wpool8, e.sb, e.sb2, e.dram = (
                e_wpool, e_wpool8, e_sb, e_sb2, e_dram)

            e.xhi = cpool.tile([128, NK, B], BF)
            nc.gpsimd.dma_start(e.xhi[:], xhi_d[:])
            e.w2t = cpool.tile([128, 2, FEAT], F32)
            nc.gpsimd.dma_start(e.w2t[:],
                                w2t_d.rearrange("(c p) f -> p c f", p=128))
            e.mw1t = cpool.tile([128, 2, HID], F32)
            nc.gpsimd.dma_start(e.mw1t[:],
                                mw1t_d.rearrange("(c p) f -> p c f", p=128))
            e.dw2 = cpool.tile([128, 2, T * FEAT], BF)
            nc.gpsimd.dma_start(e.dw2[:],
                                dw2_d.rearrange("(c p) f -> p c f", p=128))
            e.browb = cpool.tile([1, T * FEAT], BF)
            nc.gpsimd.dma_start(e.browb[:],
                                browb_d[:, FEAT + HID + T:])
            e.ones1b = cpool.tile([1, B], BF)
            nc.gpsimd.memset(e.ones1b[:], 1.0)
            e.mw2t = cpool.tile([HID, T], F32)
            nc.gpsimd.dma_start(e.mw2t[:], mw2t_d[:])
            e.brow = cpool.tile([1, FEAT + HID + T], F32)
            nc.gpsimd.dma_start(e.brow[:],
                                brow_d[:, 0:FEAT + HID + T])
            e.bias8 = cpool.tile([B, SRED], F32)
            nc.gpsimd.dma_start(e.bias8[:], bias8_d[:])
            e.id8 = cpool.tile([B, B], F32)
            nc.gpsimd.dma_start(e.id8[:], id8_d[:])
            e.ones1 = cpool.tile([1, B], F32)
            nc.gpsimd.memset(e.ones1[:], 1.0)
            e.Pmat = cpool.tile([N_CORES * B, B], BF)
            nc.gpsimd.dma_start(e.Pmat[:], pmat_d[:])

            bench = reps1 > 1 or tails > 1
            e.Sz = e.sb.tile([B, FEAT], BF, tag="Sz", name="Sz")
            e.Su = e.sb.tile([B, 4 * FEAT], BF, tag="Su", name="Su")
            with (
                tc.tile_pool(name="ps_u", bufs=1, space="PSUM") as ps_u,
                tc.tile_pool(name="ps_tp", bufs=1, space="PSUM") as e_ps_tp,
                tc.tile_pool(name="ps2", bufs=2, space="PSUM") as e_ps2,
            ):
                e.ps_tp, e.ps2 = e_ps_tp, e_ps2
                e.bk2 = ps_u.tile([128, 512], F32, tag="bk2", name="bk2")
                e.bk3 = ps_u.tile([128, 512], F32, tag="bk3", name="bk3")

                if bench:
                    with tc.tile_pool(name="ps_z", bufs=1,
                                      space="PSUM") as ps_z:
                        e.bk0 = ps_z.tile([128, 512], F32, tag="bk0",
                                          name="bk0")
                        with tc.For_i(0, reps1, 1):
                            for _bi in range(body):
                                _emit_dmas(nc, e)
                                _emit_zmms(nc, e)
                                _emit_umms(nc, e, 0, NG8)
                                _emit_Sz(nc, e)
                                _emit_Su(nc, e)
                    _emit_agz(nc, e)
                    with (
                        tc.tile_pool(name="ps_rz", bufs=1,
                                     space="PSUM") as e_ps_rz,
                        tc.tile_pool(name="ps_e", bufs=2,
                                     space="PSUM") as e_ps_e,
                    ):
                        e.ps_rz, e.ps_e = e_ps_rz, e_ps_e
                        _emit_front(nc, e)
                    # tails: repeat the EXPOSED tail (agu+back), strictly
                    # serialized via a value-neutral dep on previous o_fin
                    for it in range(tails):
                        if it > 0:
                            nc.vector.scalar_tensor_tensor(
                                e.Su[:, 0:256], e.o_fin[:], 0.0,
                                e.Su[:, 0:256], op0=AOT.mult, op1=AOT.add)
                        _emit_agu(nc, e, it=str(it))
                        with tc.tile_pool(name=f"ps_ru{it}", bufs=1,
                                          space="PSUM") as e_ps_ru:
                            e.ps_ru = e_ps_ru
                            _emit_back(nc, e, out_d)
                else:
                    with tc.tile_pool(name="ps_z", bufs=1,
                                      space="PSUM") as ps_z:
                        e.bk0 = ps_z.tile([128, 512], F32, tag="bk0",
                                          name="bk0")
                        _emit_dmas(nc, e)
                        _emit_zmms(nc, e)
                        _emit_Sz(nc, e)
                    _emit_agz(nc, e)
                    with (
                        tc.tile_pool(name="ps_rz", bufs=1,
                                     space="PSUM") as e_ps_rz,
                        tc.tile_pool(name="ps_e", bufs=2,
                                     space="PSUM") as e_ps_e,
                    ):
                        e.ps_rz, e.ps_e = e_ps_rz, e_ps_e
                        _emit_umms(nc, e, 0, 2)
                        _emit_front(nc, e)
                        _emit_umms(nc, e, 2, NG8)
                        _emit_Su(nc, e)
                        _emit_agu(nc, e)
                    with tc.tile_pool(name="ps_ru", bufs=1,
                                      space="PSUM") as e_ps_ru:
                        e.ps_ru = e_ps_ru
                        _emit_back(nc, e, out_d)

    nc.compile()
    return nc


def _get_nc(reps1=1, reps2=1, body=1, shots=1, wbufs=2, tails=1):
    key = ("nc", reps1, reps2, body, shots, wbufs, tails)
    if key not in _CACHE:
        _CACHE[key] = _build(reps1, reps2, body, shots, wbufs, tails)
    return _CACHE[key]


def _prep_inputs(x, W1, b1, W2, b2, mW1, mb1, mW2, mb2, dW1, db1, dW2, db2):
    f32 = np.float32
    xflat = np.ascontiguousarray(np.asarray(x, f32).reshape(B, D_IN))
    W1 = np.asarray(W1, f32)
    W2 = np.asarray(W2, f32)
    dW1 = np.asarray(dW1, f32)
    dW2 = np.asarray(dW2, f32)
    mW1 = np.asarray(mW1, f32)
    mW2 = np.asarray(mW2, f32)
    b1 = np.asarray(b1, f32)
    b2 = np.asarray(b2, f32)
    db1 = np.asarray(db1, f32)
    db2 = np.asarray(db2, f32)
    mb1 = np.asarray(mb1, f32)
    mb2 = np.asarray(mb2, f32)

    w2t = np.ascontiguousarray(W2.T)
    mw1t = np.ascontiguousarray(mW1.T)
    dw2cat = np.ascontiguousarray(
        np.concatenate([dW2[t].T for t in range(T)], axis=1)).astype(BF16)
    mw2t = np.ascontiguousarray(mW2.T)
    db2cat = np.concatenate([db2[t] for t in range(T)])
    brow = np.concatenate([b2, mb1, mb2, db2cat]).reshape(1, -1).astype(f32)
    bias8 = np.zeros((B, SRED), f32)
    bias8[:, 0:256] = b1 / N_CORES
    for t in range(T):
        bias8[:, 256 + 256 * t:512 + 256 * t] = db1[t] / N_CORES
    id8 = np.eye(B, dtype=f32)
    pmat = np.tile(np.eye(B, dtype=f32), (N_CORES, 1)).astype(BF16)

    in_maps = []
    for c in range(N_CORES):
        sl = slice(c * KC, (c + 1) * KC)
        wbf = (W1[:, sl].T.astype(BF16)
               .reshape(NK, 128, FEAT).transpose(1, 0, 2)
               .reshape(128, NK * FEAT))
        w8r = np.empty((KC, 4 * FEAT), dtype=FP8)
        for t in range(T):
            w8r[:, 256 * t:256 * (t + 1)] = (
                dW1[t, :, sl].T * DW1_SCALE).astype(FP8)
        w8 = (w8r.reshape(NK, 128, 4 * FEAT).transpose(1, 0, 2)
              .reshape(128, NK * 4 * FEAT))

        xc = np.ascontiguousarray(xflat[:, sl].T)
        xh = xc.astype(BF16)

        in_maps.append({
            "wbf": np.ascontiguousarray(wbf),
            "w8": np.ascontiguousarray(w8),
            "xhi": np.ascontiguousarray(
                xh.reshape(NK, 128, B).transpose(1, 0, 2)),
            "w2t": w2t,
            "mw1t": mw1t,
            "dw2cat": dw2cat,
            "mw2t": mw2t,
            "brow": brow,
            "browb": brow.astype(BF16),
            "bias8": bias8,
            "ident8": id8,
            "pmat": pmat,
        })
    return in_maps


def run(trace=False, reps1=1, reps2=1, body=1, shots=1, wbufs=2, tails=1,
        **inputs):
    nc = _get_nc(reps1, reps2, body, shots, wbufs, tails)
    in_maps = _prep_inputs(**inputs)
    res = run_bass_kernel_spmd(nc, in_maps, core_ids=list(range(N_CORES)),
                               trace=trace)
    return res.results[0]["out"].astype(np.float32), res


def kernel(**inputs) -> np.ndarray:
    import time as _time
    try:
        out, _ = run(trace=False, **inputs)
    except Exception:
        # transient device/runtime hiccups: retry once
        _time.sleep(3.0)
        out, _ = run(trace=False, **inputs)
    return out
